# revision 9
# baseline (speedup 1.0000x reference)
"""Trainium2 Bass kernel for CubicModel: out = feats(feats(x)@W0.T+b0)@W1.T+b1
where feats(z) = [z, triu(z_i z_j), z^3].

v2 strategy (8 cores, TP over the 132352-dim feature axis):
  * Layer 0 runs in fp8e4 DoubleRow matmuls with a 3-term error-compensated
    split: W*f ~= Whi*fhi + Wlo*fhi + Whi*flo, where (Whi,Wlo) are host-packed
    fp8 hi/lo halves of the prescaled weights (x256 so residuals stay out of
    the e4m3 subnormal range) and (fhi,flo) are produced on-chip from the fp16
    features (DVE/ACT convert + Pool/DVE residual subtract).  Each DoubleRow
    instruction contracts TWO 128-deep k-slices at 0.5 cycles per output
    column, so layer-0 PE time is 1.33x below the fp16 floor.
  * Layer 1 stays fp16 (feature-side engine cost makes fp8 a wash there).
  * Batch is split asymmetrically (A=320, B=192 cols): A's ReduceScatter +
    AllGather hide under B's layer-0, B's chain hides under the longer
    layer-1 A.  Bias is applied after the AllGather during the frame rebuild,
    so RS->AG run back-to-back on the Pool queue.
  * W0 (fp8, packed) is fully resident in SBUF; W0/y2x/W1 all stream through
    the SP queue in consumption order.  No final collective: each core writes
    its fp32 partial and the host sums.
"""

import sys

sys.path.insert(0, "/opt/trn_rl_repo")

import numpy as np
import ml_dtypes

E4 = ml_dtypes.float8_e4m3

N_CORES = 8
D = 512          # d_in == hidden
B = 512          # batch
H = 512          # hidden
DOUT = 256
ROT = D // N_CORES          # 64
KT = 130                    # k-tiles per core
NP = KT // 2                # 65 pairs
NQ = (NP + 1) // 2          # 33 quads (last quad holds 1 pair)
QUAD_BASE = D
CUBIC_BASE = D + (D * D + D) // 2    # 131840
N_H_TILES = H // 128        # 4
N_O_TILES = DOUT // 128     # 2
WCHUNK = 4                  # k-tiles per W1 DMA
W0CH = 5                    # pairs per W0 DMA chunk (13 chunks)
BHA = 320                   # batch cols in half A
BHB = B - BHA               # 192
GS = 256.0                  # weight prescale for fp8

# Tile schedule: same PROC order as v1 (quad tiles by ascending y2 column).
PROC = [("SQ",), ("D256",), ("L255",)]
for _col in range(32):
    for _t in (31 - _col, 63 - _col, 95 - _col, 127 - _col):
        if _t <= 126:
            PROC.append(("Q", _t))
assert len(PROC) == KT


def _triu_idx(lo, hi):
    return QUAD_BASE + lo * D - lo * (lo - 1) // 2 + (hi - lo)


def _pair_fk(i, d):
    j = (i + d) % D
    lo = np.minimum(i, j)
    hi = np.maximum(i, j)
    return _triu_idx(lo, hi)


def _core_refk(c):
    """W-row (feature) index for each tile row, in PROC order. -1 = zero pad."""
    refk = np.full((KT, 128), -1, dtype=np.int64)
    p = np.arange(64)
    base = ROT * c
    for s, desc in enumerate(PROC):
        if desc[0] == "SQ":
            i = (base + p) % D
            refk[s, 0:64] = _triu_idx(i, i)
            refk[s, 64:128] = CUBIC_BASE + i
        elif desc[0] == "D256":
            a = 32 * c + np.arange(32)
            refk[s, 0:32] = _triu_idx(a, a + 256)
        elif desc[0] == "L255":
            i = (base + p) % D
            refk[s, 0:64] = i
            refk[s, 64:128] = _pair_fk((base + 1 + p) % D, 255)
        else:
            t = desc[1]
            d1, d0 = 2 * t + 2, 2 * t + 1
            r1 = (-d1) % 64
            r0 = r1 + 1
            refk[s, 0:64] = _pair_fk((base + r1 + p) % D, d1)
            refk[s, 64:128] = _pair_fk((base + r0 + p) % D, d0)
    return refk


def _pack_y2(rows16T, b):
    """Parity-split shift array: Y2[64*(r%2)+p, r//2, :] = src[r + p]."""
    y2 = np.zeros((128, 32, b), dtype=np.float16)
    for r in range(64):
        y2[64 * (r % 2):64 * (r % 2) + 64, r // 2, :] = rows16T[r:r + 64]
    return y2


def _prep_core_inputs(c, x16T, W0T32, W1T, b0):
    refk = _core_refk(c)

    # layer-0 weights: prescale, fp8 hi/lo split, pack pairs
    w0p = np.zeros((KT, 128, H), dtype=np.float32)
    m = refk >= 0
    w0p[m] = W0T32[refk[m]] * GS
    whi = w0p.astype(E4)
    wlo = (w0p - whi.astype(np.float32)).astype(E4)
    w0b = np.zeros((NP, 128, 4, H), dtype=E4)
    for pp in range(NP):
        w0b[pp, :, 0] = whi[2 * pp]
        w0b[pp, :, 1] = wlo[2 * pp]
        w0b[pp, :, 2] = whi[2 * pp + 1]
        w0b[pp, :, 3] = wlo[2 * pp + 1]

    w1t = np.zeros((KT, 128, DOUT), dtype=np.float16)
    w1t[m] = W1T[refk[m]]

    frame = x16T[(ROT * c + np.arange(320)) % D]      # rotated 320-row frame
    y2x = _pack_y2(frame, B)

    p = np.arange(64)
    xrep = np.zeros((128, 5, B), dtype=np.float16)
    for m5 in range(5):
        xrep[0:64, m5] = frame[64 * m5 + p]
        xrep[64:128, m5] = frame[64 * m5 + p]
    xmix = np.zeros((128, 2, B), dtype=np.float16)
    xmix[0:64, :, :] = 1.0
    xmix[64:128, 0] = frame[p]
    xmix[64:128, 1] = frame[256 + p]
    xd256 = np.zeros((128, 2, B), dtype=np.float16)
    k32 = np.arange(32)
    xd256[0:32, 0] = x16T[(32 * c + k32) % D]
    xd256[0:32, 1] = x16T[(32 * c + 256 + k32) % D]

    # indirect-gather row indices for the layer-1 h-frame rebuild + bias
    pp2 = np.arange(128)
    hfridx = np.zeros((128, 4), dtype=np.int32)
    for q in range(3):
        hfridx[:, q] = (ROT * c + 128 * q + pp2) % D
    hfridx[0:32, 3] = (32 * c + k32) % D
    hfridx[32:64, 3] = (32 * c + 256 + k32) % D
    hfrb = b0.astype(np.float32)[hfridx]              # [128, 4] bias per gather row
    hfrb[64:128, 3] = 0.0

    return {
        "w0b": w0b,
        "w1t": w1t,
        "y2x": y2x,
        "xrep": xrep,
        "xmix": xmix,
        "xd256": xd256,
        "hfridx": hfridx,
        "hfrb": hfrb,
    }


def _pair_gen_info(p):
    """For pair p>=2: (y2 col m, first xrep rep r0) for the fused double-gen."""
    t0 = PROC[2 * p][1]
    t1 = PROC[2 * p + 1][1]
    m0 = 31 - t0 % 32
    m1 = 31 - t1 % 32
    r0 = t0 // 32 + 1
    r1 = t1 // 32 + 1
    assert m0 == m1 and r1 == r0 + 1, (p, t0, t1)
    return m0, r0


def _build_program():
    import concourse.mybir as mybir
    import concourse.tile as tile
    from concourse import bacc
    from concourse.bass import AP, IndirectOffsetOnAxis

    fp16 = mybir.dt.float16
    fp8 = mybir.dt.float8e4
    f32 = mybir.dt.float32
    i32 = mybir.dt.int32
    DR = mybir.MatmulPerfMode.DoubleRow
    Copy = mybir.ActivationFunctionType.Copy
    Ident = mybir.ActivationFunctionType.Identity
    CORE_IDS = list(range(N_CORES))

    nc = bacc.Bacc(None, target_bir_lowering=False, debug=False)
    with tile.TileContext(nc) as tc:
        with tc.tile_pool(name="dram", bufs=1, space="DRAM") as dram, \
             tc.tile_pool(name="const", bufs=1) as const, \
             tc.tile_pool(name="w1pool", bufs=6) as w1pool, \
             tc.tile_pool(name="spool", bufs=2) as spool, \
             tc.tile_pool(name="hapool", bufs=1) as hapool, \
             tc.tile_pool(name="ps", bufs=1, space="PSUM") as ps:
            # ---- DRAM I/O ----
            w0bd = dram.tile([NP, 128, 4, H], fp8, kind="ExternalInput", name="w0b", uniquify=False)
            w1td = dram.tile([KT, 128, DOUT], fp16, kind="ExternalInput", name="w1t", uniquify=False)
            y2xd = dram.tile([128, 32, B], fp16, kind="ExternalInput", name="y2x", uniquify=False)
            xrepd = dram.tile([128, 5, B], fp16, kind="ExternalInput", name="xrep", uniquify=False)
            xmixd = dram.tile([128, 2, B], fp16, kind="ExternalInput", name="xmix", uniquify=False)
            xd256d = dram.tile([128, 2, B], fp16, kind="ExternalInput", name="xd256", uniquify=False)
            hfridxd = dram.tile([128, 4], i32, kind="ExternalInput", name="hfridx", uniquify=False)
            hfrbd = dram.tile([128, 4], f32, kind="ExternalInput", name="hfrb", uniquify=False)
            outp = dram.tile([DOUT, B], f32, kind="ExternalOutput", name="outp", uniquify=False)

            halves = []
            for hn, lo, bh in (("A", 0, BHA), ("B", BHA, BHB)):
                halves.append({
                    "lo": lo, "hi": lo + bh, "bh": bh, "tag": hn,
                    "cc": dram.tile([H, bh], fp16, name=f"cc{hn}", uniquify=False),
                    "rs": dram.tile([ROT, bh], fp16, name=f"rs{hn}", uniquify=False),
                    "ag": dram.tile([H, bh], fp16, name=f"ag{hn}", uniquify=False,
                                    addr_space="Shared"),
                    "hfr": dram.tile([448, bh], fp16, name=f"hfr{hn}", uniquify=False),
                })

            # ---- small constants (Pool DGE) ----
            xrep_sb = const.tile([128, 5, B], fp16)
            nc.gpsimd.dma_start(out=xrep_sb, in_=xrepd[:])
            xmix_sb = const.tile([128, 2, B], fp16)
            nc.gpsimd.dma_start(out=xmix_sb, in_=xmixd[:])
            xd256_sb = const.tile([128, 2, B], fp16)
            nc.gpsimd.dma_start(out=xd256_sb, in_=xd256d[:])
            hfridx_sb = const.tile([128, 4], i32)
            nc.gpsimd.dma_start(out=hfridx_sb, in_=hfridxd[:])
            hfrb_sb = const.tile([128, 4], f32)
            nc.gpsimd.dma_start(out=hfrb_sb, in_=hfrbd[:])
            sqx = const.tile([128, B], fp16, tag="sqx")

            for hf in halves:
                hf["ps0"] = [ps.tile([128, hf["bh"]], f32, tag=f"p{hf['tag']}{h}",
                                     name=f"ps0{hf['tag']}{h}")
                             for h in range(N_H_TILES)]

            A, Bhf = halves

            with tc.tile_pool(name="w0pool", bufs=13) as w0pool, \
                 tc.tile_pool(name="y2pool", bufs=3) as y2pool, \
                 tc.tile_pool(name="fpool", bufs=2) as fpool, \
                 tc.tile_pool(name="fbpool", bufs=2) as fbpool:
                # ---- W0 stream: even chunks on SP, odd on ACT ----
                w0_sbs = {}
                n_w0ch = (NP + W0CH - 1) // W0CH
                for ci in range(n_w0ch):
                    p0 = ci * W0CH
                    npair = min(W0CH, NP - p0)
                    w_sb = w0pool.tile([128, W0CH, 4, H], fp8, tag="w0")
                    eng = nc.sync if ci % 2 == 0 else nc.scalar
                    eng.dma_start(out=w_sb[:, 0:npair], in_=w0bd[p0:p0 + npair]
                                  .rearrange("k p t h -> p k t h"))
                    w0_sbs[p0] = w_sb

                # ---- W1 streams on SP (A then B), shared ring ----
                def load_w1(sbs):
                    for s0 in range(0, KT, WCHUNK):
                        nw = min(WCHUNK, KT - s0)
                        w_sb = w1pool.tile([128, WCHUNK, DOUT], fp16, tag="w1")
                        nc.sync.dma_start(out=w_sb[:, 0:nw],
                                          in_=w1td[s0:s0 + nw].rearrange("k p h -> p k h"))
                        sbs[s0] = w_sb

                # y2x ring (chunks of 2 cols). engine differs per phase.
                def y2_chunk(sbs, cch, eng):
                    t = y2pool.tile([128, 2, B], fp16, tag="y2c")
                    eng.dma_start(out=t, in_=y2xd[:, 2 * cch:2 * cch + 2, :])
                    sbs[cch] = t

                def emit_gens(hf, q, f16q, y2_sbs, y2_eng):
                    lo, hi = hf["lo"], hf["hi"]
                    v = nc.vector
                    for j in (0, 1):
                        p = 2 * q + j
                        if p >= NP:
                            break
                        bh = hf["bh"]
                        if p == 0:
                            v.tensor_mul(sqx[:, lo:hi], xrep_sb[:, 0, lo:hi],
                                         xrep_sb[:, 0, lo:hi])
                            v.tensor_mul(f16q[:, 0, 0:bh], sqx[:, lo:hi],
                                         xmix_sb[:, 0, lo:hi])
                            v.tensor_mul(f16q[:, 1, 0:bh], xd256_sb[:, 0, lo:hi],
                                         xd256_sb[:, 1, lo:hi])
                        elif p == 1:
                            if 0 not in y2_sbs:
                                y2_chunk(y2_sbs, 0, y2_eng)
                            y2c = y2_sbs[0]
                            v.tensor_mul(f16q[:, 2, 0:bh], y2c[:, 0, lo:hi],
                                         xmix_sb[:, 1, lo:hi])
                            v.tensor_mul(f16q[:, 3, 0:bh], y2c[:, 0, lo:hi],
                                         xrep_sb[:, 1, lo:hi])
                        else:
                            m, r0 = _pair_gen_info(p)
                            if m // 2 not in y2_sbs:
                                y2_chunk(y2_sbs, m // 2, y2_eng)
                            y2c = y2_sbs[m // 2]
                            bh = hf["bh"]
                            in1 = AP(y2c.tensor,
                                     y2c.offset + (m % 2) * B + lo,
                                     [[2 * B, 128], [0, 2], [1, bh]])
                            v.tensor_mul(f16q[:, 2 * j:2 * j + 2, 0:bh],
                                         xrep_sb[:, r0:r0 + 2, lo:hi], in1)

                def emit_quad(hf, q, y2_sbs, y2_eng, conv_eng, sub_eng):
                    bh = hf["bh"]
                    npair = min(2, NP - 2 * q)
                    nsl = 2 * npair
                    f16q = fpool.tile([128, 4, BHA], fp16, tag="f16")
                    fb = fbpool.tile([128, 8, BHA], fp8, tag="fb")
                    emit_gens(hf, q, f16q, y2_sbs, y2_eng)
                    hi_ap = AP(fb.tensor, fb.offset + BHA,
                               [[8 * BHA, 128], [2 * BHA, nsl], [1, bh]])
                    lo_ap = AP(fb.tensor, fb.offset,
                               [[8 * BHA, 128], [2 * BHA, nsl], [1, bh]])
                    f16full = f16q[:, 0:nsl, 0:bh]
                    if conv_eng is nc.scalar:
                        conv_eng.activation(hi_ap, f16full, Copy)
                    else:
                        conv_eng.tensor_copy(hi_ap, f16full)
                    sub_eng.tensor_sub(lo_ap, f16full, hi_ap)
                    # matmuls
                    mchunks = [(0, 256), (256, bh)] if bh > 256 else [(0, bh)]
                    for j in range(npair):
                        p = 2 * q + j
                        w_sb = w0_sbs[(p // W0CH) * W0CH]
                        iw = p % W0CH
                        wbase = w_sb.offset + iw * (4 * H)
                        fbase = fb.offset + 4 * j * BHA
                        first = (p == 0)
                        last = (p == NP - 1)
                        for h in range(N_H_TILES):
                            for term, (wo, ws, fo, fs) in enumerate((
                                    (0, 2 * H, BHA, 2 * BHA),     # main: Whi x fhi pair
                                    (0, H, 0, BHA),               # corr k1
                                    (2 * H, H, 2 * BHA, BHA))):   # corr k2
                                st = AP(w_sb.tensor, wbase + wo + h * 128,
                                        [[W0CH * 4 * H, 128], [ws, 2], [1, 128]])
                                for mi, (c0, c1) in enumerate(mchunks):
                                    mv = AP(fb.tensor, fbase + fo + c0,
                                            [[8 * BHA, 128], [fs, 2], [1, c1 - c0]])
                                    nc.tensor.matmul(
                                        hf["ps0"][h][:, c0:c1], st, mv,
                                        start=(first and term == 0 and mi == 0),
                                        stop=(last and term == 2 and mi == len(mchunks) - 1),
                                        perf_mode=DR)

                def emit_evac(hf):
                    stage = spool.tile([128, N_H_TILES, BHA], fp16, tag="evac")
                    bh = hf["bh"]
                    for h in range(N_H_TILES):
                        nc.scalar.activation(stage[:, h, 0:bh], hf["ps0"][h], Copy,
                                             scale=1.0 / GS)
                        nc.scalar.dma_start(out=hf["cc"][128 * h:128 * h + 128, :],
                                            in_=stage[:, h, 0:bh])

                def emit_chain(hf):
                    nc.gpsimd.collective_compute(
                        "ReduceScatter", mybir.AluOpType.add,
                        replica_groups=[CORE_IDS], ins=[hf["cc"][:]], outs=[hf["rs"][:]],
                    )
                    nc.gpsimd.collective_compute(
                        "AllGather", mybir.AluOpType.bypass,
                        replica_groups=[CORE_IDS], ins=[hf["rs"][:]], outs=[hf["ag"][:]],
                    )

                def emit_tail(hf, hpool):
                    """Gather rotated h-frame from ag, add bias, build L1 operands."""
                    bh = hf["bh"]
                    hfs = hpool.tile([128, 4, bh], fp16, tag="hfs")
                    for q in range(3):
                        nc.gpsimd.indirect_dma_start(
                            out=hfs[:, q, :], out_offset=None, in_=hf["ag"][:],
                            in_offset=IndirectOffsetOnAxis(ap=hfridx_sb[:, q:q + 1], axis=0),
                        )
                    nc.gpsimd.indirect_dma_start(
                        out=hfs[0:64, 3, :], out_offset=None, in_=hf["ag"][:],
                        in_offset=IndirectOffsetOnAxis(ap=hfridx_sb[0:64, 3:4], axis=0),
                    )
                    hfsb = hpool.tile([128, 4, bh], fp16, tag="hfsb")
                    for q in range(3):
                        nc.scalar.activation(hfsb[:, q, :], hfs[:, q, :], Ident,
                                             bias=hfrb_sb[:, q:q + 1])
                    nc.scalar.activation(hfsb[0:64, 3, :], hfs[0:64, 3, :], Ident,
                                         bias=hfrb_sb[0:64, 3:4])
                    hfrd = hf["hfr"]
                    for q in range(3):
                        nc.scalar.dma_start(out=hfrd[128 * q:128 * q + 128, :],
                                            in_=hfsb[:, q, :])
                    nc.scalar.dma_start(out=hfrd[384:448, :], in_=hfsb[0:64, 3, :])

                    def win(row0, pn, sn, sstride_rows):
                        base = hfrd[row0:row0 + 1, :]
                        return AP(base.tensor, base.offset,
                                  [[bh, pn], [sstride_rows * bh, sn], [1, bh]])

                    hrep = hpool.tile([128, 5, bh], fp16, tag="hrep")
                    nc.scalar.dma_start(out=hrep[0:64, :, :], in_=win(0, 64, 5, 64))
                    nc.scalar.dma_start(out=hrep[64:128, :, :], in_=win(0, 64, 5, 64))
                    hmix = hpool.tile([128, 2, bh], fp16, tag="hmix")
                    nc.vector.memset(hmix[0:64, :, :], 1.0)
                    nc.scalar.dma_start(out=hmix[64:128, :, :], in_=win(0, 64, 2, 256))
                    hd256 = hpool.tile([128, 2, bh], fp16, tag="hd256")
                    nc.vector.memset(hd256[32:64, :, :], 0.0)
                    nc.vector.memset(hd256[64:128, :, :], 0.0)
                    nc.scalar.dma_start(out=hd256[0:32, :, :], in_=win(384, 32, 2, 32))
                    y2h = hpool.tile([128, 32, bh], fp16, tag="y2h")
                    for (s0, s1) in [(0, 8), (8, 32)]:
                        nc.scalar.dma_start(out=y2h[0:64, s0:s1, :],
                                            in_=win(2 * s0, 64, s1 - s0, 2))
                        nc.scalar.dma_start(out=y2h[64:128, s0:s1, :],
                                            in_=win(2 * s0 + 1, 64, s1 - s0, 2))
                    sqh = hpool.tile([128, bh], fp16, tag="sqh")
                    hf["h_ops"] = (y2h, hrep, hmix, hd256, sqh)
                    hf["ps1"] = [ps.tile([128, hf["bh"]], f32, tag=f"p{hf['tag']}{h}",
                                         name=f"ps1{hf['tag']}{h}")
                                 for h in range(N_O_TILES)]

                # ================= layer 0 =================
                w1a_sbs = {}
                w1b_sbs = {}
                y2a_sbs = {}
                y2b_sbs = {}
                # half A: conv on DVE early (ACT busy with W0-odd), ACT later; sub Pool
                for q in range(NQ):
                    conv_eng = nc.vector if q < 18 else nc.scalar
                    emit_quad(A, q, y2a_sbs, nc.sync, conv_eng, nc.gpsimd)
                emit_evac(A)
                emit_chain(A)
                emit_tail(A, hapool)
                # half B: conv ACT, sub DVE (Pool owns A's collectives)
                for q in range(NQ):
                    emit_quad(Bhf, q, y2b_sbs, nc.sync, nc.scalar, nc.vector)
                emit_evac(Bhf)
                # W1 streams (SP queue, after y2x-B chunks)
                load_w1(w1a_sbs)
                load_w1(w1b_sbs)
                emit_chain(Bhf)

            # ---- inner pools closed: W0/y2x/f16/fb space freed ----
            with tc.tile_pool(name="hbpool", bufs=1) as hbpool:
                emit_tail(Bhf, hbpool)

                def emit_gen1(hf, s, ft):
                    y2h, hrep, hmix, hd256, sqh = hf["h_ops"]
                    desc = PROC[s]
                    v = nc.vector
                    if desc[0] == "SQ":
                        v.tensor_mul(sqh[:, :], hrep[:, 0, :], hrep[:, 0, :])
                        v.tensor_mul(ft[:, :], sqh[:, :], hmix[:, 0, :])
                    elif desc[0] == "D256":
                        v.tensor_mul(ft[:, :], hd256[:, 0, :], hd256[:, 1, :])
                    elif desc[0] == "L255":
                        v.tensor_mul(ft[:, :], y2h[:, 0, :], hmix[:, 1, :])
                    else:
                        t = desc[1]
                        v.tensor_mul(ft[:, :], y2h[:, 31 - (t % 32), :],
                                     hrep[:, t // 32 + 1, :])

                def emit_l1(hf, s, w_sbs, l1pool):
                    ft = l1pool.tile([128, hf["bh"]], fp16, tag="ft")
                    emit_gen1(hf, s, ft)
                    w_sb = w_sbs[(s // WCHUNK) * WCHUNK]
                    kk = s % WCHUNK
                    for h in range(N_O_TILES):
                        nc.tensor.matmul(hf["ps1"][h],
                                         w_sb[:, kk, 128 * h:128 * h + 128], ft,
                                         start=(s == 0), stop=(s == KT - 1))

                def emit_out(hf):
                    lo, hi = hf["lo"], hf["hi"]
                    for h in range(N_O_TILES):
                        so = spool.tile([128, BHA], f32, tag="oevac")
                        nc.scalar.activation(so[:, 0:hf["bh"]], hf["ps1"][h], Copy)
                        nc.gpsimd.dma_start(out=outp[128 * h:128 * h + 128, lo:hi],
                                            in_=so[:, 0:hf["bh"]])

                with tc.tile_pool(name="l1pool", bufs=3) as l1pool:
                    for s in range(KT):
                        emit_l1(A, s, w1a_sbs, l1pool)
                    emit_out(A)
                    for s in range(KT):
                        emit_l1(Bhf, s, w1b_sbs, l1pool)
                    emit_out(Bhf)
    nc.compile()
    return nc


_NC_CACHE = None


def build_in_maps(x, W0, b0, W1, b1):
    x16T = np.ascontiguousarray(x.T).astype(np.float16)          # [D, B]
    W0T32 = np.ascontiguousarray(W0.T).astype(np.float32)        # [K, H]
    W1T = np.ascontiguousarray(W1.T).astype(np.float16)          # [K, DOUT]
    return [_prep_core_inputs(c, x16T, W0T32, W1T, b0) for c in range(N_CORES)]


def kernel(x, W0, b0, W1, b1):
    global _NC_CACHE
    from concourse.bass_utils import run_bass_kernel_spmd

    in_maps = build_in_maps(x, W0, b0, W1, b1)
    if _NC_CACHE is None:
        _NC_CACHE = _build_program()
    res = run_bass_kernel_spmd(_NC_CACHE, in_maps, list(range(N_CORES)))
    acc = np.zeros((DOUT, B), dtype=np.float32)
    for c in range(N_CORES):
        acc += res.results[c]["outp"]
    acc += b1.astype(np.float32).reshape(DOUT, 1)
    return np.ascontiguousarray(acc.T)


# revision 21
# speedup vs baseline: 1.2569x; 1.2569x over previous
"""Trainium2 Bass kernel for CubicModel: out = feats(feats(x)@W0.T+b0)@W1.T+b1
where feats(z) = [z, triu(z_i z_j), z^3].

v2 strategy (8 cores, TP over the 132352-dim feature axis):
  * Layer 0 runs in fp8e4 DoubleRow matmuls with a 3-term error-compensated
    split: W*f ~= Whi*fhi + Wlo*fhi + Whi*flo, where (Whi,Wlo) are host-packed
    fp8 hi/lo halves of the prescaled weights (x256 so residuals stay out of
    the e4m3 subnormal range) and (fhi,flo) are produced on-chip from the fp16
    features (DVE/ACT convert + Pool/DVE residual subtract).  Each DoubleRow
    instruction contracts TWO 128-deep k-slices at 0.5 cycles per output
    column, so layer-0 PE time is 1.33x below the fp16 floor.
  * Layer 1 stays fp16 (feature-side engine cost makes fp8 a wash there).
  * Batch is split asymmetrically (A=320, B=192 cols): A's ReduceScatter +
    AllGather hide under B's layer-0, B's chain hides under the longer
    layer-1 A.  Bias is applied after the AllGather during the frame rebuild,
    so RS->AG run back-to-back on the Pool queue.
  * W0 (fp8, packed) is fully resident in SBUF; W0/y2x/W1 all stream through
    the SP queue in consumption order.  No final collective: each core writes
    its fp32 partial and the host sums.
"""

import sys

sys.path.insert(0, "/opt/trn_rl_repo")

import numpy as np
import ml_dtypes

E4 = ml_dtypes.float8_e4m3

N_CORES = 8
D = 512          # d_in == hidden
B = 512          # batch
H = 512          # hidden
DOUT = 256
ROT = D // N_CORES          # 64
KT = 130                    # k-tiles per core
NP = KT // 2                # 65 pairs
NQ = (NP + 1) // 2          # 33 quads (last quad holds 1 pair)
QUAD_BASE = D
CUBIC_BASE = D + (D * D + D) // 2    # 131840
N_H_TILES = H // 128        # 4
N_O_TILES = DOUT // 128     # 2
WCHUNK = 4                  # k-tiles per W1 DMA
W0CH = 5                    # pairs per W0 DMA chunk (13 chunks)
BHA = 320                   # batch cols in half A
BHB = B - BHA               # 192
GS = 256.0                  # weight prescale for fp8

# Tile schedule: same PROC order as v1 (quad tiles by ascending y2 column).
PROC = [("SQ",), ("D256",), ("L255",)]
for _col in range(32):
    for _t in (31 - _col, 63 - _col, 95 - _col, 127 - _col):
        if _t <= 126:
            PROC.append(("Q", _t))
assert len(PROC) == KT


def _triu_idx(lo, hi):
    return QUAD_BASE + lo * D - lo * (lo - 1) // 2 + (hi - lo)


def _pair_fk(i, d):
    j = (i + d) % D
    lo = np.minimum(i, j)
    hi = np.maximum(i, j)
    return _triu_idx(lo, hi)


def _core_refk(c):
    """W-row (feature) index for each tile row, in PROC order. -1 = zero pad."""
    refk = np.full((KT, 128), -1, dtype=np.int64)
    p = np.arange(64)
    base = ROT * c
    for s, desc in enumerate(PROC):
        if desc[0] == "SQ":
            i = (base + p) % D
            refk[s, 0:64] = _triu_idx(i, i)
            refk[s, 64:128] = CUBIC_BASE + i
        elif desc[0] == "D256":
            a = 32 * c + np.arange(32)
            refk[s, 0:32] = _triu_idx(a, a + 256)
        elif desc[0] == "L255":
            i = (base + p) % D
            refk[s, 0:64] = i
            refk[s, 64:128] = _pair_fk((base + 1 + p) % D, 255)
        else:
            t = desc[1]
            d1, d0 = 2 * t + 2, 2 * t + 1
            r1 = (-d1) % 64
            r0 = r1 + 1
            refk[s, 0:64] = _pair_fk((base + r1 + p) % D, d1)
            refk[s, 64:128] = _pair_fk((base + r0 + p) % D, d0)
    return refk


def _pack_y2(rows16T, b):
    """Parity-split shift array: Y2[64*(r%2)+p, r//2, :] = src[r + p]."""
    y2 = np.zeros((128, 32, b), dtype=np.float16)
    for r in range(64):
        y2[64 * (r % 2):64 * (r % 2) + 64, r // 2, :] = rows16T[r:r + 64]
    return y2


def _prep_core_inputs(c, x16T, W0T32, W1T, b0):
    refk = _core_refk(c)

    # layer-0 weights: prescale, fp8 hi/lo split, pack pairs
    w0p = np.zeros((KT, 128, H), dtype=np.float32)
    m = refk >= 0
    w0p[m] = W0T32[refk[m]] * GS
    whi = w0p.astype(E4)
    wlo = (w0p - whi.astype(np.float32)).astype(E4)
    w0b = np.zeros((NP, 128, 4, H), dtype=E4)
    for pp in range(NP):
        w0b[pp, :, 0] = whi[2 * pp]
        w0b[pp, :, 1] = wlo[2 * pp]
        w0b[pp, :, 2] = whi[2 * pp + 1]
        w0b[pp, :, 3] = wlo[2 * pp + 1]

    w1t = np.zeros((KT, 128, DOUT), dtype=np.float16)
    w1t[m] = W1T[refk[m]]

    frame = x16T[(ROT * c + np.arange(320)) % D]      # rotated 320-row frame
    y2x = _pack_y2(frame, B)

    p = np.arange(64)
    xrep = np.zeros((128, 5, B), dtype=np.float16)
    for m5 in range(5):
        xrep[0:64, m5] = frame[64 * m5 + p]
        xrep[64:128, m5] = frame[64 * m5 + p]
    xmix = np.zeros((128, 2, B), dtype=np.float16)
    xmix[0:64, :, :] = 1.0
    xmix[64:128, 0] = frame[p]
    xmix[64:128, 1] = frame[256 + p]
    xd256 = np.zeros((128, 2, B), dtype=np.float16)
    k32 = np.arange(32)
    xd256[0:32, 0] = x16T[(32 * c + k32) % D]
    xd256[0:32, 1] = x16T[(32 * c + 256 + k32) % D]

    # indirect-gather row indices for the layer-1 h-frame rebuild + bias
    pp2 = np.arange(128)
    hfridx = np.zeros((128, 4), dtype=np.int32)
    for q in range(3):
        hfridx[:, q] = (ROT * c + 128 * q + pp2) % D
    hfridx[0:32, 3] = (32 * c + k32) % D
    hfridx[32:64, 3] = (32 * c + 256 + k32) % D
    hfrb = b0.astype(np.float32)[hfridx]              # [128, 4] bias per gather row
    hfrb[64:128, 3] = 0.0

    return {
        "w0b": w0b,
        "w1t": w1t,
        "y2x": y2x,
        "xrep": xrep,
        "xmix": xmix,
        "xd256": xd256,
        "hfridx": hfridx,
        "hfrb": hfrb,
    }


def _pair_gen_info(p):
    """For pair p>=2: (y2 col m, first xrep rep r0) for the fused double-gen."""
    t0 = PROC[2 * p][1]
    t1 = PROC[2 * p + 1][1]
    m0 = 31 - t0 % 32
    m1 = 31 - t1 % 32
    r0 = t0 // 32 + 1
    r1 = t1 // 32 + 1
    assert m0 == m1 and r1 == r0 + 1, (p, t0, t1)
    return m0, r0


def _build_program():
    import concourse.mybir as mybir
    import concourse.tile as tile
    from concourse import bacc
    from concourse.bass import AP, IndirectOffsetOnAxis

    fp16 = mybir.dt.float16
    fp8 = mybir.dt.float8e4
    f32 = mybir.dt.float32
    i32 = mybir.dt.int32
    DR = mybir.MatmulPerfMode.DoubleRow
    Copy = mybir.ActivationFunctionType.Copy
    Ident = mybir.ActivationFunctionType.Identity
    CORE_IDS = list(range(N_CORES))

    nc = bacc.Bacc(None, target_bir_lowering=False, debug=False)
    with tile.TileContext(nc) as tc:
        with tc.tile_pool(name="dram", bufs=1, space="DRAM") as dram, \
             tc.tile_pool(name="const", bufs=1) as const, \
             tc.tile_pool(name="w1pool", bufs=6) as w1pool, \
             tc.tile_pool(name="spool", bufs=1) as spool, \
             tc.tile_pool(name="hapool", bufs=1) as hapool, \
             tc.tile_pool(name="ps", bufs=1, space="PSUM") as ps:
            # ---- DRAM I/O ----
            w0bd = dram.tile([NP, 128, 4, H], fp8, kind="ExternalInput", name="w0b", uniquify=False)
            w1td = dram.tile([KT, 128, DOUT], fp16, kind="ExternalInput", name="w1t", uniquify=False)
            y2xd = dram.tile([128, 32, B], fp16, kind="ExternalInput", name="y2x", uniquify=False)
            xrepd = dram.tile([128, 5, B], fp16, kind="ExternalInput", name="xrep", uniquify=False)
            xmixd = dram.tile([128, 2, B], fp16, kind="ExternalInput", name="xmix", uniquify=False)
            xd256d = dram.tile([128, 2, B], fp16, kind="ExternalInput", name="xd256", uniquify=False)
            hfridxd = dram.tile([128, 4], i32, kind="ExternalInput", name="hfridx", uniquify=False)
            hfrbd = dram.tile([128, 4], f32, kind="ExternalInput", name="hfrb", uniquify=False)
            outp = dram.tile([DOUT, B], f32, kind="ExternalOutput", name="outp", uniquify=False)

            halves = []
            for hn, lo, bh in (("A", 0, BHA), ("B", BHA, BHB)):
                halves.append({
                    "lo": lo, "hi": lo + bh, "bh": bh, "tag": hn,
                    "cc": dram.tile([H, bh], fp16, name=f"cc{hn}", uniquify=False),
                    "rs": dram.tile([ROT, bh], fp16, name=f"rs{hn}", uniquify=False),
                    "ag": dram.tile([H, bh], fp16, name=f"ag{hn}", uniquify=False,
                                    addr_space="Shared"),
                    "hfr": dram.tile([448, bh], fp16, name=f"hfr{hn}", uniquify=False),
                })

            # ---- small constants (Pool DGE) ----
            xrep_sb = const.tile([128, 5, B], fp16)
            nc.gpsimd.dma_start(out=xrep_sb, in_=xrepd[:])
            xmix_sb = const.tile([128, 2, B], fp16)
            nc.gpsimd.dma_start(out=xmix_sb, in_=xmixd[:])
            xd256_sb = const.tile([128, 2, B], fp16)
            nc.gpsimd.dma_start(out=xd256_sb, in_=xd256d[:])
            hfridx_sb = const.tile([128, 4], i32)
            nc.gpsimd.dma_start(out=hfridx_sb, in_=hfridxd[:])
            hfrb_sb = const.tile([128, 4], f32)
            nc.gpsimd.dma_start(out=hfrb_sb, in_=hfrbd[:])
            sqx = const.tile([128, B], fp16, tag="sqx")

            for hf in halves:
                hf["ps0"] = [ps.tile([128, hf["bh"]], f32, tag=f"p{hf['tag']}{h}",
                                     name=f"ps0{hf['tag']}{h}")
                             for h in range(N_H_TILES)]

            A, Bhf = halves

            with tc.tile_pool(name="w0pool", bufs=1) as w0pool, \
                 tc.tile_pool(name="y2pool", bufs=2) as y2pool, \
                 tc.tile_pool(name="fpool", bufs=3) as fpool, \
                 tc.tile_pool(name="fbpool", bufs=3) as fbpool:
                # ---- W0 stream (fully resident): small first chunks for fast
                # start, then alternating SP / ACT queues ----
                w0_sbs = {}
                w0_chunks = [1, 2] + [W0CH] * 13
                w0_chunks[-1] = NP - sum(w0_chunks[:-1])
                p0 = 0
                for ci, npair in enumerate(w0_chunks):
                    w_sb = w0pool.tile([128, npair, 4, H], fp8, tag=f"w0c{ci}")
                    eng = nc.sync if ci % 2 == 0 else nc.scalar
                    eng.dma_start(out=w_sb, in_=w0bd[p0:p0 + npair]
                                  .rearrange("k p t h -> p k t h"))
                    for pp in range(p0, p0 + npair):
                        w0_sbs[pp] = (w_sb, pp - p0)
                    p0 += npair

                # ---- W1 stream helper (pool + engine chosen per half) ----
                def load_w1(sbs, pool, eng, tagp):
                    for s0 in range(0, KT, WCHUNK):
                        nw = min(WCHUNK, KT - s0)
                        tg = "w1" if tagp is None else f"{tagp}{s0}"
                        w_sb = pool.tile([128, WCHUNK, DOUT], fp16, tag=tg)
                        eng.dma_start(out=w_sb[:, 0:nw],
                                      in_=w1td[s0:s0 + nw].rearrange("k p h -> p k h"))
                        sbs[s0] = w_sb

                # y2x ring (chunks of 2 cols). engine differs per phase.
                def y2_chunk(sbs, cch, eng):
                    t = y2pool.tile([128, 2, B], fp16, tag="y2c")
                    eng.dma_start(out=t, in_=y2xd[:, 2 * cch:2 * cch + 2, :])
                    sbs[cch] = t

                def emit_gens(hf, q, f16q, y2_sbs, y2_eng):
                    lo, hi = hf["lo"], hf["hi"]
                    v = nc.vector
                    for j in (0, 1):
                        p = 2 * q + j
                        if p >= NP:
                            break
                        bh = hf["bh"]
                        if p == 0:
                            v.tensor_mul(sqx[:, lo:hi], xrep_sb[:, 0, lo:hi],
                                         xrep_sb[:, 0, lo:hi])
                            v.tensor_mul(f16q[:, 0, 0:bh], sqx[:, lo:hi],
                                         xmix_sb[:, 0, lo:hi])
                            v.tensor_mul(f16q[:, 1, 0:bh], xd256_sb[:, 0, lo:hi],
                                         xd256_sb[:, 1, lo:hi])
                        elif p == 1:
                            if 0 not in y2_sbs:
                                y2_chunk(y2_sbs, 0, y2_eng)
                            y2c = y2_sbs[0]
                            v.tensor_mul(f16q[:, 2, 0:bh], y2c[:, 0, lo:hi],
                                         xmix_sb[:, 1, lo:hi])
                            v.tensor_mul(f16q[:, 3, 0:bh], y2c[:, 0, lo:hi],
                                         xrep_sb[:, 1, lo:hi])
                        else:
                            m, r0 = _pair_gen_info(p)
                            if m // 2 not in y2_sbs:
                                y2_chunk(y2_sbs, m // 2, y2_eng)
                            y2c = y2_sbs[m // 2]
                            bh = hf["bh"]
                            in1 = AP(y2c.tensor,
                                     y2c.offset + (m % 2) * B + lo,
                                     [[2 * B, 128], [0, 2], [1, bh]])
                            v.tensor_mul(f16q[:, 2 * j:2 * j + 2, 0:bh],
                                         xrep_sb[:, r0:r0 + 2, lo:hi], in1)

                def emit_quad(hf, q, y2_sbs, y2_eng, conv_eng, sub_eng):
                    bh = hf["bh"]
                    npair = min(2, NP - 2 * q)
                    nsl = 2 * npair
                    f16q = fpool.tile([128, 4, BHA], fp16, tag="f16")
                    fb = fbpool.tile([128, 8, BHA], fp8, tag="fb")
                    emit_gens(hf, q, f16q, y2_sbs, y2_eng)
                    hi_ap = AP(fb.tensor, fb.offset + BHA,
                               [[8 * BHA, 128], [2 * BHA, nsl], [1, bh]])
                    lo_ap = AP(fb.tensor, fb.offset,
                               [[8 * BHA, 128], [2 * BHA, nsl], [1, bh]])
                    f16full = f16q[:, 0:nsl, 0:bh]
                    if conv_eng is nc.scalar:
                        conv_eng.activation(hi_ap, f16full, Copy)
                    else:
                        conv_eng.tensor_copy(hi_ap, f16full)
                    sub_eng.tensor_sub(lo_ap, f16full, hi_ap)
                    # matmuls
                    mchunks = [(0, 256), (256, bh)] if bh > 256 else [(0, bh)]
                    for j in range(npair):
                        p = 2 * q + j
                        w_sb, iw = w0_sbs[p]
                        wpstride = w_sb.shape[1] * 4 * H
                        wbase = w_sb.offset + iw * (4 * H)
                        fbase = fb.offset + 4 * j * BHA
                        first = (p == 0)
                        last = (p == NP - 1)
                        for h in range(N_H_TILES):
                            for term, (wo, ws, fo, fs) in enumerate((
                                    (0, 2 * H, BHA, 2 * BHA),     # main: Whi x fhi pair
                                    (0, H, 0, BHA),               # corr k1
                                    (2 * H, H, 2 * BHA, BHA))):   # corr k2
                                st = AP(w_sb.tensor, wbase + wo + h * 128,
                                        [[wpstride, 128], [ws, 2], [1, 128]])
                                for mi, (c0, c1) in enumerate(mchunks):
                                    mv = AP(fb.tensor, fbase + fo + c0,
                                            [[8 * BHA, 128], [fs, 2], [1, c1 - c0]])
                                    nc.tensor.matmul(
                                        hf["ps0"][h][:, c0:c1], st, mv,
                                        start=(first and term == 0 and mi == 0),
                                        stop=(last and term == 2 and mi == len(mchunks) - 1),
                                        perf_mode=DR)

                def emit_evac(hf):
                    stage = spool.tile([128, N_H_TILES, BHA], fp16, tag="evac")
                    bh = hf["bh"]
                    for h in range(N_H_TILES):
                        nc.scalar.activation(stage[:, h, 0:bh], hf["ps0"][h], Copy,
                                             scale=1.0 / GS)
                        nc.scalar.dma_start(out=hf["cc"][128 * h:128 * h + 128, :],
                                            in_=stage[:, h, 0:bh])

                def emit_chain(hf):
                    nc.gpsimd.collective_compute(
                        "ReduceScatter", mybir.AluOpType.add,
                        replica_groups=[CORE_IDS], ins=[hf["cc"][:]], outs=[hf["rs"][:]],
                    )
                    nc.gpsimd.collective_compute(
                        "AllGather", mybir.AluOpType.bypass,
                        replica_groups=[CORE_IDS], ins=[hf["rs"][:]], outs=[hf["ag"][:]],
                    )

                def emit_gather(hf, hpool):
                    """Indirect-gather the rotated h-frame rows from ag (Pool)."""
                    bh = hf["bh"]
                    hfs = hpool.tile([128, 4, bh], fp16, tag="hfs")
                    for q in range(3):
                        nc.gpsimd.indirect_dma_start(
                            out=hfs[:, q, :], out_offset=None, in_=hf["ag"][:],
                            in_offset=IndirectOffsetOnAxis(ap=hfridx_sb[:, q:q + 1], axis=0),
                        )
                    nc.gpsimd.indirect_dma_start(
                        out=hfs[0:64, 3, :], out_offset=None, in_=hf["ag"][:],
                        in_offset=IndirectOffsetOnAxis(ap=hfridx_sb[0:64, 3:4], axis=0),
                    )
                    hf["hfs"] = hfs

                def emit_tail(hf, hpool):
                    """Bias the gathered rows, write frame, build L1 operands (ACT)."""
                    bh = hf["bh"]
                    hfs = hf["hfs"]
                    hfsb = hpool.tile([128, 4, bh], fp16, tag="hfsb")
                    for q in range(3):
                        nc.scalar.activation(hfsb[:, q, :], hfs[:, q, :], Ident,
                                             bias=hfrb_sb[:, q:q + 1])
                    nc.scalar.activation(hfsb[0:64, 3, :], hfs[0:64, 3, :], Ident,
                                         bias=hfrb_sb[0:64, 3:4])
                    hfrd = hf["hfr"]
                    for q in range(3):
                        nc.scalar.dma_start(out=hfrd[128 * q:128 * q + 128, :],
                                            in_=hfsb[:, q, :])
                    nc.scalar.dma_start(out=hfrd[384:448, :], in_=hfsb[0:64, 3, :])

                    def win(row0, pn, sn, sstride_rows):
                        base = hfrd[row0:row0 + 1, :]
                        return AP(base.tensor, base.offset,
                                  [[bh, pn], [sstride_rows * bh, sn], [1, bh]])

                    hrep = hpool.tile([128, 5, bh], fp16, tag="hrep")
                    nc.scalar.dma_start(out=hrep[0:64, :, :], in_=win(0, 64, 5, 64))
                    nc.scalar.dma_start(out=hrep[64:128, :, :], in_=win(0, 64, 5, 64))
                    hmix = hpool.tile([128, 2, bh], fp16, tag="hmix")
                    nc.vector.memset(hmix[0:64, :, :], 1.0)
                    nc.scalar.dma_start(out=hmix[64:128, :, :], in_=win(0, 64, 2, 256))
                    hd256 = hpool.tile([128, 2, bh], fp16, tag="hd256")
                    nc.vector.memset(hd256[32:64, :, :], 0.0)
                    nc.vector.memset(hd256[64:128, :, :], 0.0)
                    nc.scalar.dma_start(out=hd256[0:32, :, :], in_=win(384, 32, 2, 32))
                    y2h = hpool.tile([128, 32, bh], fp16, tag="y2h")
                    for (s0, s1) in [(0, 4), (4, 12), (12, 32)]:
                        nc.scalar.dma_start(out=y2h[0:64, s0:s1, :],
                                            in_=win(2 * s0, 64, s1 - s0, 2))
                        nc.scalar.dma_start(out=y2h[64:128, s0:s1, :],
                                            in_=win(2 * s0 + 1, 64, s1 - s0, 2))
                    sqh = hpool.tile([128, bh], fp16, tag="sqh")
                    hf["h_ops"] = (y2h, hrep, hmix, hd256, sqh)
                    hf["ps1"] = [ps.tile([128, hf["bh"]], f32, tag=f"p{hf['tag']}{h}",
                                         name=f"ps1{hf['tag']}{h}")
                                 for h in range(N_O_TILES)]

                # ================= layer 0 =================
                w1a_sbs = {}
                w1b_sbs = {}
                y2a_sbs = {}
                y2b_sbs = {}
                # half A: y2x on Pool DGE; conv on DVE early (ACT busy with
                # W0-odd), ACT later; sub on Pool
                for q in range(NQ):
                    conv_eng = nc.vector if q < 18 else nc.scalar
                    emit_quad(A, q, y2a_sbs, nc.gpsimd, conv_eng, nc.gpsimd)
                emit_evac(A)
                emit_chain(A)
                emit_gather(A, hapool)        # Pool is idle during half B
                # half B: y2x on SP (ahead of W1-A!); conv ACT, sub DVE
                for q in range(NQ):
                    emit_quad(Bhf, q, y2b_sbs, nc.sync, nc.scalar, nc.vector)
                load_w1(w1a_sbs, w1pool, nc.sync, None)   # SP: after y2x-B
                emit_evac(Bhf)
                emit_tail(A, hapool)          # ACT: after evac-B
                emit_chain(Bhf)

            # ---- inner pools closed: W0/y2x/f16/fb space freed ----
            with tc.tile_pool(name="hbpool", bufs=1) as hbpool, \
                 tc.tile_pool(name="w1bpool", bufs=1) as w1bpool:
                emit_gather(Bhf, hbpool)
                # W1-B fully resident, loaded through the Pool-queue hole
                # after AG-B / B-gathers (space WAR-frees when W0 is done)
                load_w1(w1b_sbs, w1bpool, nc.gpsimd, "w1b")
                emit_tail(Bhf, hbpool)

                def emit_gen1(hf, s, ft):
                    y2h, hrep, hmix, hd256, sqh = hf["h_ops"]
                    desc = PROC[s]
                    v = nc.vector
                    if desc[0] == "SQ":
                        v.tensor_mul(sqh[:, :], hrep[:, 0, :], hrep[:, 0, :])
                        v.tensor_mul(ft[:, :], sqh[:, :], hmix[:, 0, :])
                    elif desc[0] == "D256":
                        v.tensor_mul(ft[:, :], hd256[:, 0, :], hd256[:, 1, :])
                    elif desc[0] == "L255":
                        v.tensor_mul(ft[:, :], y2h[:, 0, :], hmix[:, 1, :])
                    else:
                        t = desc[1]
                        v.tensor_mul(ft[:, :], y2h[:, 31 - (t % 32), :],
                                     hrep[:, t // 32 + 1, :])

                def emit_l1(hf, s, w_sbs, l1pool):
                    ft = l1pool.tile([128, hf["bh"]], fp16, tag="ft")
                    emit_gen1(hf, s, ft)
                    w_sb = w_sbs[(s // WCHUNK) * WCHUNK]
                    kk = s % WCHUNK
                    for h in range(N_O_TILES):
                        nc.tensor.matmul(hf["ps1"][h],
                                         w_sb[:, kk, 128 * h:128 * h + 128], ft,
                                         start=(s == 0), stop=(s == KT - 1))

                def emit_out(hf):
                    lo, hi = hf["lo"], hf["hi"]
                    for h in range(N_O_TILES):
                        so = spool.tile([128, BHA], f32, tag="oevac")
                        nc.scalar.activation(so[:, 0:hf["bh"]], hf["ps1"][h], Copy)
                        nc.gpsimd.dma_start(out=outp[128 * h:128 * h + 128, lo:hi],
                                            in_=so[:, 0:hf["bh"]])

                with tc.tile_pool(name="l1pool", bufs=3) as l1pool:
                    for s in range(KT):
                        emit_l1(A, s, w1a_sbs, l1pool)
                    emit_out(A)
                    for s in range(KT):
                        emit_l1(Bhf, s, w1b_sbs, l1pool)
                    emit_out(Bhf)
    nc.compile()
    return nc


_NC_CACHE = None


def build_in_maps(x, W0, b0, W1, b1):
    x16T = np.ascontiguousarray(x.T).astype(np.float16)          # [D, B]
    W0T32 = np.ascontiguousarray(W0.T).astype(np.float32)        # [K, H]
    W1T = np.ascontiguousarray(W1.T).astype(np.float16)          # [K, DOUT]
    return [_prep_core_inputs(c, x16T, W0T32, W1T, b0) for c in range(N_CORES)]


def kernel(x, W0, b0, W1, b1):
    global _NC_CACHE
    from concourse.bass_utils import run_bass_kernel_spmd

    in_maps = build_in_maps(x, W0, b0, W1, b1)
    if _NC_CACHE is None:
        _NC_CACHE = _build_program()
    res = run_bass_kernel_spmd(_NC_CACHE, in_maps, list(range(N_CORES)))
    acc = np.zeros((DOUT, B), dtype=np.float32)
    for c in range(N_CORES):
        acc += res.results[c]["outp"]
    acc += b1.astype(np.float32).reshape(DOUT, 1)
    return np.ascontiguousarray(acc.T)
